# revision 1
# baseline (speedup 1.0000x reference)
"""Trainium2 Bass kernel for nn_CrossAggregator (gnn_message_passing).

out[g,o] = self[g]·W1[o,:] + ea_g^T A_o eb_g,  g=(b,m), A_o = W[o,128:].reshape(128,128)
ea/eb = masked means over 32 neighbors (t=0 / t=1).

Design (per core, batch/8 data-parallel, G=512 rows):
- ea-side: masked-mean + partition-broadcast FUSED into K=32 f32r matmuls
  (stationary = all-ones/32, row-group selected via tile_position) -> PSUM earep.
- eb-side: masked-mean via K=128 f32r matmuls with a banded selector (BIG) as
  stationary -> ebT [j,g] directly in PSUM.
- mask multiplies on GPSIMD (tensor_tensor, stride-0 mask broadcast), f32r out.
- outer-product chunks Pt on DVE: pt[j, (i,g)] = ebT[j,g] * earep_i[j,g], f32r out.
- main contraction on PE: psum_out[o,g] += W2chunk_i^T @ pt_i  (f32r, N=512).
- host does only layout transforms (shard/permute/pack) + output transpose.
"""
import sys
import numpy as np

for _p in ("/opt/trn_rl_repo", "/root/.axon_site/_ro/trn_rl_repo"):
    if _p not in sys.path:
        sys.path.insert(0, _p)

B, M, TWO, NN, D = 1024, 4, 2, 32, 128
OUT = 128
NCORES = 8
BC = B // NCORES          # batches per core
G = BC * M                # 512 rows per core
NIG = D // 4              # 32 slabs of 4 i's

_CACHE = {}


def _build_nc():
    import os
    import concourse.bacc as bacc_mod
    import concourse.mybir as mybir
    from concourse.tile import TileContext

    F32 = mybir.dt.float32
    F32R = mybir.dt.float32r
    MUL = mybir.AluOpType.mult

    nc = bacc_mod.Bacc(None)
    SKIP_GPS = bool(int(os.environ.get("SKIP_GPS", "0")))
    SKIP_PT = bool(int(os.environ.get("SKIP_PT", "0")))
    SKIP_MAIN = bool(int(os.environ.get("SKIP_MAIN", "0")))

    d_naR = nc.declare_dram_parameter("naR", [NIG, 128, G], F32, isOutput=False)
    d_nbR = nc.declare_dram_parameter("nbR", [NIG, 128, G], F32, isOutput=False)
    d_maskA = nc.declare_dram_parameter("maskA", [128, G], F32, isOutput=False)
    d_maskB = nc.declare_dram_parameter("maskB", [128, G], F32, isOutput=False)
    d_selfT = nc.declare_dram_parameter("selfT", [D, G], F32, isOutput=False)
    d_W1 = nc.declare_dram_parameter("W1a", [D, OUT], F32, isOutput=False)
    d_W2 = nc.declare_dram_parameter("W2p", [NIG, D, 4 * OUT], F32, isOutput=False)
    d_BIG = nc.declare_dram_parameter("BIG", [128, 252], F32, isOutput=False)
    d_ones = nc.declare_dram_parameter("ones32", [128, 128], F32, isOutput=False)
    d_out = nc.declare_dram_parameter("outT", [OUT, G], F32, isOutput=True)

    with TileContext(nc) as tc:
        with (
            tc.tile_pool(name="const", bufs=1) as cpool,
            tc.tile_pool(name="nb_raw", bufs=2) as nbpool,
            tc.tile_pool(name="nb_msk", bufs=2) as nbmpool,
            tc.tile_pool(name="na_raw", bufs=2) as napool,
            tc.tile_pool(name="na_msk", bufs=2) as nampool,
            tc.tile_pool(name="w2", bufs=4) as w2pool,
            tc.tile_pool(name="pt", bufs=2) as ptpool,
            tc.tile_pool(name="misc", bufs=1) as mpool,
            tc.tile_pool(name="ps_ebt", bufs=1, space="PSUM") as ps_ebt,
            tc.tile_pool(name="ps_rep", bufs=2, space="PSUM") as ps_rep,
            tc.tile_pool(name="ps_out", bufs=1, space="PSUM") as ps_out,
        ):
            # constants
            big_t = cpool.tile([128, 252], F32R, tag="big")
            nc.sync.dma_start(out=big_t[:], in_=d_BIG[:].bitcast(F32R))
            ones_t = cpool.tile([128, 128], F32R, tag="ones")
            nc.sync.dma_start(out=ones_t[:], in_=d_ones[:].bitcast(F32R))
            maskA_t = cpool.tile([128, G], F32, tag="ma")
            nc.sync.dma_start(out=maskA_t[:], in_=d_maskA[:])
            maskB_t = cpool.tile([128, G], F32, tag="mb")
            nc.sync.dma_start(out=maskB_t[:], in_=d_maskB[:])
            selfT_t = cpool.tile([D, G], F32R, tag="sT")
            nc.sync.dma_start(out=selfT_t[:], in_=d_selfT[:].bitcast(F32R))
            w1_t = cpool.tile([D, OUT], F32R, tag="w1")
            nc.sync.dma_start(out=w1_t[:], in_=d_W1[:].bitcast(F32R))

            _loop_n = int(os.environ.get("KERNEL_LOOP", "0"))
            _amp = int(os.environ.get("KERNEL_AMP", "1"))
            from contextlib import nullcontext
            _ctx = tc.For_i(0, _loop_n, 1) if _loop_n else nullcontext()
            with _ctx:
              for _rep in range(_amp):
                # ---- EB phase: ebT[j, g] in PSUM ----
                p_ebt = ps_ebt.tile([128, G], F32, tag="ebt")
                for sg in range(8):  # 4 slabs per DMA
                    nb4 = nbpool.tile([128, 4 * G], F32R if SKIP_GPS else F32, tag="nb4")
                    nc.sync.dma_start(
                        out=nb4[:].rearrange("p (s c) -> p s c", s=4),
                        in_=d_nbR[4 * sg : 4 * sg + 4].bitcast(F32R if SKIP_GPS else F32).rearrange("s p c -> p s c"),
                    )
                    if SKIP_GPS:
                        mb4 = nb4
                    else:
                        mb4 = nbmpool.tile([128, 4 * G], F32R, tag="mb4")
                        nc.gpsimd.tensor_tensor(
                            out=mb4[:].rearrange("p (s c) -> p s c", s=4),
                            in0=nb4[:].rearrange("p (s c) -> p s c", s=4),
                            in1=maskB_t[:][:, None, :].broadcast_to([128, 4, G]),
                            op=MUL,
                        )
                    for u in range(4):
                        jg = 4 * sg + u
                        nc.tensor.matmul(
                            p_ebt[:],
                            big_t[:, 124 - 4 * jg : 252 - 4 * jg],
                            mb4[:, G * u : G * (u + 1)],
                            start=(jg == 0),
                            stop=(jg == NIG - 1),
                        )
                ebT_sb = mpool.tile([128, G], F32, tag="ebsb")
                nc.scalar.copy(out=ebT_sb[:], in_=p_ebt[:])

                # ---- MAIN phase ----
                p_out = ps_out.tile([OUT, G], F32, tag="out")
                nc.tensor.matmul(p_out[:], w1_t[:], selfT_t[:], start=True, stop=False)

                ma2_tiles = {}
                for k in range(64):  # pair k covers i = 2k, 2k+1
                    ig = k // 2
                    if ig % 2 == 0 and k % 2 == 0:
                        na2 = napool.tile([128, 2 * G], F32R if SKIP_GPS else F32, tag="na2")
                        nc.sync.dma_start(
                            out=na2[:].rearrange("p (s c) -> p s c", s=2),
                            in_=d_naR[ig : ig + 2].bitcast(F32R if SKIP_GPS else F32).rearrange("s p c -> p s c"),
                        )
                        if SKIP_GPS:
                            ma2 = na2
                        else:
                            ma2 = nampool.tile([128, 2 * G], F32R, tag="ma2")
                            nc.gpsimd.tensor_tensor(
                                out=ma2[:].rearrange("p (s c) -> p s c", s=2),
                                in0=na2[:].rearrange("p (s c) -> p s c", s=2),
                                in1=maskA_t[:][:, None, :].broadcast_to([128, 2, G]),
                                op=MUL,
                            )
                        ma2_tiles[ig] = ma2
                        ma2_tiles[ig + 1] = ma2
                    if k % 2 == 0:
                        w2_t = w2pool.tile([D, 4 * OUT], F32R, tag="w2t")
                        nc.sync.dma_start(
                            out=w2_t[:], in_=d_W2[k // 2].bitcast(F32R)
                        )
                    ma2 = ma2_tiles[ig]
                    slab_off = (ig % 2) * G  # which slab within the pair tile
                    rep = ps_rep.tile([128, 2 * G], F32, tag="rep")
                    for u in range(2):
                        isub = 2 * (k % 2) + u
                        nc.tensor.matmul(
                            rep[:, G * u : G * (u + 1)],
                            ones_t[32 * isub : 32 * isub + 32, :],
                            ma2[32 * isub : 32 * isub + 32, slab_off : slab_off + G],
                            start=True,
                            stop=True,
                            tile_position=(32 * isub, 0),
                        )
                    if SKIP_PT:
                        pt2 = ma2
                    else:
                        pt2 = ptpool.tile([128, 2 * G], F32R, tag="pt2")
                        if k % int(os.environ.get("PTMOD", "4")) == int(os.environ.get("PTMOD", "4")) - 1 and not SKIP_GPS:
                            rep_sb = ptpool.tile([128, 2 * G], F32, tag="repsb")
                            nc.scalar.copy(out=rep_sb[:], in_=rep[:])
                            nc.gpsimd.tensor_tensor(
                                out=pt2[:].rearrange("p (a c) -> p a c", a=2),
                                in0=ebT_sb[:][:, None, :].broadcast_to([128, 2, G]),
                                in1=rep_sb[:].rearrange("p (a c) -> p a c", a=2),
                                op=MUL,
                            )
                        else:
                            nc.vector.tensor_tensor(
                                out=pt2[:].rearrange("p (a c) -> p a c", a=2),
                                in0=ebT_sb[:][:, None, :].broadcast_to([128, 2, G]),
                                in1=rep[:].rearrange("p (a c) -> p a c", a=2),
                                op=MUL,
                            )
                    for u in (range(0) if SKIP_MAIN else range(2)):
                        i = 2 * k + u
                        w2col = (i % 4) * OUT
                        nc.tensor.matmul(
                            p_out[:],
                            w2_t[:, w2col : w2col + OUT],
                            pt2[:, G * u : G * (u + 1)],
                            start=False,
                            stop=(k == 63 and u == 1),
                        )

                out_sb = mpool.tile([OUT, G], F32, tag="osb")
                nc.scalar.copy(out=out_sb[:], in_=p_out[:])
                nc.sync.dma_start(out=d_out[:], in_=out_sb[:])

    nc.finalize()
    return nc


def _host_prep(self_vectors, neighbor_vectors, masks, W):
    f32 = np.float32
    sv = np.ascontiguousarray(self_vectors, dtype=f32)
    nv = np.ascontiguousarray(neighbor_vectors, dtype=f32)
    mk = np.ascontiguousarray(masks, dtype=f32)
    Wf = np.ascontiguousarray(W, dtype=f32)

    # per-core packs
    nvc = nv.reshape(NCORES, G, TWO, NN, D)          # [c, g, t, n, d]
    naR = np.ascontiguousarray(
        nvc[:, :, 0].transpose(0, 3, 2, 1).reshape(NCORES, NIG, 128, G)
    )  # [c, ig, (isub,n), g]
    nbR = np.ascontiguousarray(
        nvc[:, :, 1].transpose(0, 3, 2, 1).reshape(NCORES, NIG, 128, G)
    )
    mkc = mk.reshape(NCORES, G, TWO, NN)             # [c, g, t, n]
    mA = mkc[:, :, 0].transpose(0, 2, 1)             # [c, n, g]
    mB = mkc[:, :, 1].transpose(0, 2, 1)
    maskA = np.ascontiguousarray(
        np.broadcast_to(mA[:, None], (NCORES, 4, NN, G)).reshape(NCORES, 128, G)
    )
    maskB = np.ascontiguousarray(
        np.broadcast_to(mB[:, None], (NCORES, 4, NN, G)).reshape(NCORES, 128, G)
    )
    selfT = np.ascontiguousarray(
        sv.reshape(NCORES, G, D).transpose(0, 2, 1)
    )  # [c, d, g]

    # shared weights
    W1a = np.ascontiguousarray(Wf[:, :D].T)                       # [d, o]
    w2 = Wf[:, D:].reshape(OUT, D, D)                             # [o, i, j]
    W2p = np.ascontiguousarray(
        w2.transpose(1, 2, 0)                                     # [i, j, o]
        .reshape(NIG, 4, D, OUT)                                  # [ig, isub, j, o]
        .transpose(0, 2, 1, 3)                                    # [ig, j, isub, o]
        .reshape(NIG, D, 4 * OUT)
    )
    BIG = np.zeros((128, 252), f32)
    r = np.arange(128)
    BIG[r, 124 + r // 32] = 1.0 / 32.0
    ones32 = np.full((128, 128), 1.0 / 32.0, f32)

    in_maps = []
    for c in range(NCORES):
        in_maps.append(
            {
                "naR": naR[c],
                "nbR": nbR[c],
                "maskA": maskA[c],
                "maskB": maskB[c],
                "selfT": selfT[c],
                "W1a": W1a,
                "W2p": W2p,
                "BIG": BIG,
                "ones32": ones32,
            }
        )
    return in_maps


def kernel(self_vectors, neighbor_vectors, masks, W, b):
    from concourse.bass_utils import run_bass_kernel_spmd

    if "nc" not in _CACHE:
        _CACHE["nc"] = _build_nc()
    nc = _CACHE["nc"]
    in_maps = _host_prep(self_vectors, neighbor_vectors, masks, W)
    results = run_bass_kernel_spmd(nc, in_maps, list(range(NCORES))).results
    out = np.empty((B, M, OUT), np.float32)
    for c in range(NCORES):
        out[c * BC : (c + 1) * BC] = (
            results[c]["outT"].T.reshape(BC, M, OUT)
        )
    out += np.asarray(b, np.float32)[None, None, :]
    return out



# revision 13
# speedup vs baseline: 1.1548x; 1.1548x over previous
"""Trainium2 Bass kernel for nn_CrossAggregator (gnn_message_passing).

out[g,o] = self[g]·W1[o,:] + ea_g^T A_o eb_g,  g=(b,m), A_o = W[o,128:].reshape(128,128)
ea/eb = masked means over 32 neighbors (t=0 / t=1).

Design (per core, batch/8 data-parallel, G=512 rows), all heavy data bf16:
- inputs packed on host into partition-major mega-tensors; DMA'd in 8-slab
  chunks (few, large DMAs: HWDGE generation was the f32 baseline's bottleneck).
- eb-side: masked-mean via K=128 bf16 matmuls with a banded selector (BIG) as
  stationary -> ebT [j,g] in PSUM; copied to SBUF as bf16.
- ea-side: masked-mean + partition-broadcast fused into bf16 matmuls
  (stationary = all-ones/32, row-group selected via tile_position) -> PSUM rep.
- mask multiplies on GPSIMD (tensor_tensor, stride-0 mask broadcast), bf16.
- outer-product chunks Pt on DVE: pt[j,(i,g)] = ebT[j,g] * rep_i[j,g], bf16 out.
- main contraction on PE: psum_out[o,g] += W2chunk_i^T @ pt_i (bf16, f32 acc).
- host does only layout transforms (shard/permute/pack/bf16 cast) + out transpose.
"""
import sys
import numpy as np

for _p in ("/opt/trn_rl_repo", "/root/.axon_site/_ro/trn_rl_repo"):
    if _p not in sys.path:
        sys.path.insert(0, _p)

B, M, TWO, NN, D = 1024, 4, 2, 32, 128
OUT = 128
NCORES = 8
BC = B // NCORES          # batches per core
G = BC * M                # 512 rows per core
NIG = D // 4              # 32 slabs of 4 j's (partition packing (q,n))
CH = 8                    # slabs per DMA chunk -> 4 chunks per side
NCHUNK = NIG // CH

_CACHE = {}


def _build_nc():
    import os
    import concourse.bacc as bacc_mod
    import concourse.mybir as mybir
    from concourse.tile import TileContext

    F32 = mybir.dt.float32
    BF16 = mybir.dt.bfloat16
    MUL = mybir.AluOpType.mult

    nc = bacc_mod.Bacc(None)

    d_naA = nc.declare_dram_parameter("naA", [128, NIG * G], BF16, isOutput=False)
    d_nbA = nc.declare_dram_parameter("nbA", [128, NIG * G], BF16, isOutput=False)
    d_maskA = nc.declare_dram_parameter("maskA", [128, G], BF16, isOutput=False)
    d_maskB = nc.declare_dram_parameter("maskB", [128, G], BF16, isOutput=False)
    d_selfT = nc.declare_dram_parameter("selfT", [D, G], BF16, isOutput=False)
    d_W1 = nc.declare_dram_parameter("W1a", [D, OUT], BF16, isOutput=False)
    d_W2 = nc.declare_dram_parameter("W2A", [D, NIG * 4 * OUT], BF16, isOutput=False)
    d_BIG = nc.declare_dram_parameter("BIG", [128, 252], BF16, isOutput=False)
    d_ones = nc.declare_dram_parameter("ones32", [128, 128], BF16, isOutput=False)
    d_out = nc.declare_dram_parameter("outT", [OUT, G], F32, isOutput=True)

    with TileContext(nc) as tc:
        with (
            tc.tile_pool(name="const", bufs=1) as cpool,
            tc.tile_pool(name="nb_raw", bufs=2) as nbpool,
            tc.tile_pool(name="nb_msk", bufs=2) as nbmpool,
            tc.tile_pool(name="na_raw", bufs=2) as napool,
            tc.tile_pool(name="na_msk", bufs=2) as nampool,
            tc.tile_pool(name="w2", bufs=2) as w2pool,
            tc.tile_pool(name="pt", bufs=3) as ptpool,
            tc.tile_pool(name="misc", bufs=1) as mpool,
            tc.tile_pool(name="ps_ebt", bufs=1, space="PSUM") as ps_ebt,
            tc.tile_pool(name="ps_rep", bufs=3, space="PSUM") as ps_rep,
            tc.tile_pool(name="ps_out", bufs=1, space="PSUM") as ps_out,
        ):
            # constants / small tensors
            big_t = cpool.tile([128, 252], BF16, tag="big")
            nc.sync.dma_start(out=big_t[:], in_=d_BIG[:])
            ones_t = cpool.tile([128, 128], BF16, tag="ones")
            nc.sync.dma_start(out=ones_t[:], in_=d_ones[:])
            maskA_t = cpool.tile([128, G], BF16, tag="ma")
            nc.sync.dma_start(out=maskA_t[:], in_=d_maskA[:])
            maskB_t = cpool.tile([128, G], BF16, tag="mb")
            nc.sync.dma_start(out=maskB_t[:], in_=d_maskB[:])
            selfT_t = cpool.tile([D, G], BF16, tag="sT")
            nc.sync.dma_start(out=selfT_t[:], in_=d_selfT[:])
            w1_t = cpool.tile([D, OUT], BF16, tag="w1")
            nc.sync.dma_start(out=w1_t[:], in_=d_W1[:])

            _loop_n = int(os.environ.get("KERNEL_LOOP", "0"))
            _amp = int(os.environ.get("KERNEL_AMP", "1"))
            from contextlib import nullcontext
            _ctx = tc.For_i(0, _loop_n, 1) if _loop_n else nullcontext()
            with _ctx:
              for _rep in range(_amp):
                # chunk emission: na DMA + DVE mask + W2 DMA for chunk c
                ma_tiles = {}
                w2_tiles = {}

                na_tiles = {}

                def emit_chunk_na_dma(c):
                    na_t = napool.tile([128, CH * G], BF16, tag="na")
                    nc.sync.dma_start(
                        out=na_t[:], in_=d_naA[:, c * CH * G : (c + 1) * CH * G]
                    )
                    na_tiles[c] = na_t

                def emit_chunk_mask(c):
                    na_t = na_tiles.pop(c)
                    ma_t = nampool.tile([128, CH * G], BF16, tag="ma8")
                    nc.vector.tensor_tensor(
                        out=ma_t[:].rearrange("p (s c) -> p s c", s=CH),
                        in0=na_t[:].rearrange("p (s c) -> p s c", s=CH),
                        in1=maskA_t[:][:, None, :].broadcast_to([128, CH, G]),
                        op=MUL,
                    )
                    ma_tiles[c] = ma_t

                def emit_chunk_w2(c):
                    w2_t = w2pool.tile([128, CH * 4 * OUT], BF16, tag="w2t")
                    nc.sync.dma_start(
                        out=w2_t[:],
                        in_=d_W2[:, c * CH * 4 * OUT : (c + 1) * CH * 4 * OUT],
                    )
                    w2_tiles[c] = w2_t

                def emit_chunk(c):
                    emit_chunk_na_dma(c)
                    emit_chunk_mask(c)
                    emit_chunk_w2(c)

                rep_tiles = {}

                def emit_rep(k):
                    ig = k // 2
                    s = ig % CH
                    ma_t = ma_tiles[ig // CH]
                    rep = ps_rep.tile([128, 2 * G], F32, tag="rep")
                    for u in range(2):
                        isub = 2 * (k % 2) + u
                        nc.tensor.matmul(
                            rep[:, G * u : G * (u + 1)],
                            ones_t[32 * isub : 32 * isub + 32, :],
                            ma_t[32 * isub : 32 * isub + 32, s * G : (s + 1) * G],
                            start=True,
                            stop=True,
                            tile_position=(32 * isub, 0),
                        )
                    rep_tiles[k] = rep

                # ---- EB phase: ebT[j, g] in PSUM via banded-selector matmuls ----
                # mask multiplies on DVE (bf16 all-SBUF -> 4x perf mode);
                # chunk-0 rep matmuls interleaved to fill PE while nb streams.
                p_ebt = ps_ebt.tile([128, G], F32, tag="ebt")
                nb_tiles = []
                for c in range(NCHUNK):
                    nb_t = nbpool.tile([128, CH * G], BF16, tag="nb")
                    nc.sync.dma_start(
                        out=nb_t[:], in_=d_nbA[:, c * CH * G : (c + 1) * CH * G]
                    )
                    if c == 0:
                        emit_chunk_na_dma(0)  # na0 DMA right behind nb0
                    mb_t = nbmpool.tile([128, CH * G], BF16, tag="mb8")
                    nc.vector.tensor_tensor(
                        out=mb_t[:].rearrange("p (s c) -> p s c", s=CH),
                        in0=nb_t[:].rearrange("p (s c) -> p s c", s=CH),
                        in1=maskB_t[:][:, None, :].broadcast_to([128, CH, G]),
                        op=MUL,
                    )
                    for u in range(CH):
                        jg = c * CH + u
                        nc.tensor.matmul(
                            p_ebt[:],
                            big_t[:, 124 - 4 * jg : 252 - 4 * jg],
                            mb_t[:, G * u : G * (u + 1)],
                            start=(jg == 0),
                            stop=(jg == NIG - 1),
                        )
                    if c == 1:
                        emit_chunk_mask(0)  # ma0 mask after mb0/mb1 on DVE
                    if c >= 2:
                        emit_rep(c - 2)  # rep k=0..1 while eb still streaming
                emit_chunk_w2(0)  # W2-0 DMA behind the whole nb stream
                ebT_sb = mpool.tile([128, G], BF16, tag="ebsb")
                nc.scalar.copy(out=ebT_sb[:], in_=p_ebt[:])

                # ---- MAIN phase ----
                p_out = ps_out.tile([OUT, G], F32, tag="out")
                nc.tensor.matmul(p_out[:], w1_t[:], selfT_t[:], start=True, stop=False)

                # pt scheduling: 'v' = DVE direct from PSUM (f32 in, 1x rate)
                #                'c' = Act copy to bf16 SBUF, Pool multiply
                #                'x' = Act copy to bf16 SBUF, DVE multiply (2x)
                ptsched = os.environ.get("PT_SCHED", "vcxvxcvxvcxvxcvx")
                pt_tiles = {}

                def emit_pt(k):
                    rep = rep_tiles.pop(k)
                    pt2 = ptpool.tile([128, 2 * G], BF16, tag="pt2")
                    mode = ptsched[k % len(ptsched)]
                    if mode == "v":
                        nc.vector.tensor_tensor(
                            out=pt2[:].rearrange("p (a c) -> p a c", a=2),
                            in0=ebT_sb[:][:, None, :].broadcast_to([128, 2, G]),
                            in1=rep[:].rearrange("p (a c) -> p a c", a=2),
                            op=MUL,
                        )
                    else:
                        rep_sb = ptpool.tile([128, 2 * G], BF16, tag="repsb")
                        nc.scalar.copy(out=rep_sb[:], in_=rep[:])
                        eng = nc.gpsimd if mode == "c" else nc.vector
                        eng.tensor_tensor(
                            out=pt2[:].rearrange("p (a c) -> p a c", a=2),
                            in0=ebT_sb[:][:, None, :].broadcast_to([128, 2, G]),
                            in1=rep_sb[:].rearrange("p (a c) -> p a c", a=2),
                            op=MUL,
                        )
                    pt_tiles[k] = pt2

                emit_pt(0)
                for k in range(64):  # pair k covers i = 2k, 2k+1 ; ig = k//2
                    ig = k // 2
                    c, s = ig // CH, ig % CH
                    if k % 16 == 6 and c + 1 < NCHUNK:
                        emit_chunk(c + 1)  # chunk-ahead prefetch
                    if k + 2 <= 63 and (k + 2) not in rep_tiles and (k + 2) not in pt_tiles:
                        emit_rep(k + 2)  # rep two steps ahead of main_k
                    if k + 1 <= 63 and (k + 1) not in pt_tiles:
                        emit_pt(k + 1)  # pt one step ahead of main_k
                    pt2 = pt_tiles.pop(k)
                    w2_t = w2_tiles[c]
                    for u in range(2):
                        isub = 2 * (k % 2) + u
                        nc.tensor.matmul(
                            p_out[:],
                            w2_t[:, s * 4 * OUT + isub * OUT : s * 4 * OUT + (isub + 1) * OUT],
                            pt2[:, G * u : G * (u + 1)],
                            start=False,
                            stop=(k == 63 and u == 1),
                        )

                out_sb = mpool.tile([OUT, G], F32, tag="osb")
                nc.scalar.copy(out=out_sb[:], in_=p_out[:])
                nc.sync.dma_start(out=d_out[:], in_=out_sb[:])

    nc.finalize()
    return nc


def _host_prep(self_vectors, neighbor_vectors, masks, W):
    import ml_dtypes

    f32 = np.float32
    bf16 = ml_dtypes.bfloat16
    sv = np.asarray(self_vectors, dtype=f32)
    nv = np.asarray(neighbor_vectors, dtype=f32)
    mk = np.asarray(masks, dtype=f32)
    Wf = np.asarray(W, dtype=f32)

    # per-core packs: partition p = (q, n) holds feature j = 4*ig + q
    # cols = (ig, g)
    nvc = nv.reshape(NCORES, G, TWO, NN, D)          # [c, g, t, n, d]

    def pack_side(t):
        arr = nvc[:, :, t]                            # [c, g, n, d]
        arr = arr.transpose(0, 3, 2, 1)               # [c, d, n, g]
        arr = arr.reshape(NCORES, NIG, 4, NN, G)      # [c, ig, q, n, g]
        arr = arr.transpose(0, 2, 3, 1, 4)            # [c, q, n, ig, g]
        return np.ascontiguousarray(
            arr.reshape(NCORES, 128, NIG * G).astype(bf16)
        )

    naA = pack_side(0)
    nbA = pack_side(1)

    mkc = mk.reshape(NCORES, G, TWO, NN)             # [c, g, t, n]
    mA = mkc[:, :, 0].transpose(0, 2, 1)             # [c, n, g]
    mB = mkc[:, :, 1].transpose(0, 2, 1)
    maskA = np.ascontiguousarray(
        np.broadcast_to(mA[:, None], (NCORES, 4, NN, G)).reshape(NCORES, 128, G).astype(bf16)
    )
    maskB = np.ascontiguousarray(
        np.broadcast_to(mB[:, None], (NCORES, 4, NN, G)).reshape(NCORES, 128, G).astype(bf16)
    )
    selfT = np.ascontiguousarray(
        sv.reshape(NCORES, G, D).transpose(0, 2, 1).astype(bf16)
    )  # [c, d, g]

    # shared weights
    W1a = np.ascontiguousarray(Wf[:, :D].T.astype(bf16))          # [d, o]
    w2 = Wf[:, D:].reshape(OUT, D, D)                             # [o, i, j]
    W2A = np.ascontiguousarray(
        w2.transpose(2, 1, 0)                                     # [j, i, o]
        .reshape(D, NIG, 4, OUT)                                  # [j, ig, isub, o]
        .reshape(D, NIG * 4 * OUT)
        .astype(bf16)
    )
    BIG = np.zeros((128, 252), f32)
    r = np.arange(128)
    BIG[r, 124 + r // 32] = 1.0 / 32.0
    BIG = BIG.astype(bf16)
    ones32 = np.full((128, 128), 1.0 / 32.0, bf16)

    in_maps = []
    for c in range(NCORES):
        in_maps.append(
            {
                "naA": naA[c],
                "nbA": nbA[c],
                "maskA": maskA[c],
                "maskB": maskB[c],
                "selfT": selfT[c],
                "W1a": W1a,
                "W2A": W2A,
                "BIG": BIG,
                "ones32": ones32,
            }
        )
    return in_maps


def kernel(self_vectors, neighbor_vectors, masks, W, b):
    from concourse.bass_utils import run_bass_kernel_spmd

    if "nc" not in _CACHE:
        _CACHE["nc"] = _build_nc()
    nc = _CACHE["nc"]
    in_maps = _host_prep(self_vectors, neighbor_vectors, masks, W)
    results = run_bass_kernel_spmd(nc, in_maps, list(range(NCORES))).results
    out = np.empty((B, M, OUT), np.float32)
    for c in range(NCORES):
        out[c * BC : (c + 1) * BC] = (
            results[c]["outT"].T.reshape(BC, M, OUT)
        )
    out += np.asarray(b, np.float32)[None, None, :]
    return out
